# revision 31
# baseline (speedup 1.0000x reference)
"""3-layer GCN (message passing) on 8 Trainium2 NeuronCores.

Strategy (graph/data parallel, per sharding hint):
  - Nodes sharded by destination across 8 cores (6250 dst rows each);
    edges bucketed by dst owner on the host; weights replicated.
  - Per layer:  out = Ahat @ (z @ W^T) + b  ==  (Ahat @ z) @ W^T + b
    where Ahat = D^-1/2 (A+I) D^-1/2.  Each core computes its dst shard:
      1. real edges: gather z[src] rows (fp16) for its edges via
         dma_gather (4 SWDGE queues) from a full local fp16 replica of z,
      2. self-loops: sequential dma_start of the core's own shard rows
         (no gather indices needed; one-hot uses dinv^2 diag columns),
      3. scatter-add into 256-dst PSUM groups via one-hot matmul
         (one-hot built on DVE: (iota == dst_local) * norm),
      4. dense W^T matmul (feature-major), bias+ReLU on ACT,
      5. transpose to node-major and store the shard,
      6. AllGather the fp16 shards -> full z for the next layer.
  - PSUM->SBUF copies and bias adds run on the ACT engine so DVE does
    only the one-hot builds.
  - Graph prep (degrees, norms, edge bucketing/padding) is host-side.
"""
import logging
import math
import re

import numpy as np

import concourse.bass as bass
import concourse.tile as tile
from concourse import bacc, mybir

N = 50000
E = 600000
D = 128
N_CORES = 8
SHARD = N // N_CORES          # 6250
GW = 256                      # dst-group width (psum group)
N_GROUPS = math.ceil(SHARD / GW)   # 25 (24*256 + 106)
HALF = N // 2                 # gather-table halves (int16 index limit)
SHARD_PAD = 6272              # self-loop block overread pad (24*256+128)
IDX_PER_CALL = 1024
BLK = 128
F16 = mybir.dt.float16
F32 = mybir.dt.float32
I16 = mybir.dt.int16


# ---------------------------------------------------------------- host prep

def _wrap_idx(flat):
    """dma_gather index layout: [128, S/16] int16, idx i at [i%16, i//16],
    replicated across the 8 gpsimd 16-partition groups."""
    S = flat.shape[0]
    arr = np.zeros((128, S // 16), np.int16)
    w = flat.reshape(S // 16, 16).T          # [16, S/16]
    for grp in range(8):
        arr[grp * 16:(grp + 1) * 16, :] = w
    return arr


def chunk_layout(nchunks, gw=GW):
    """Chunked zfull layout: chunk j holds [core0 rows, core1 rows, ...] for
    a contiguous range of dst groups, so each chunk's AllGather is one
    contiguous in/out slice and can fire as soon as its groups are stored.
    Returns (group_starts, chunk_rows, chunk_base) with per-chunk group
    ranges, per-core row counts, and zfull base offsets."""
    n_groups = math.ceil(SHARD / gw)
    gpc = math.ceil(n_groups / nchunks)
    group_starts = list(range(0, n_groups, gpc))
    chunk_rows, chunk_base = [], []
    base = 0
    for j, gs in enumerate(group_starts):
        ge = min(gs + gpc, n_groups)
        rows = min(ge * gw, SHARD) - gs * gw
        chunk_rows.append(rows)
        chunk_base.append(base)
        base += N_CORES * rows
    return group_starts, chunk_rows, chunk_base


def node_pos(nchunks, gw=GW):
    """Position of each node in the chunked zfull layout ([N] int64)."""
    if nchunks <= 1:
        return np.arange(N, dtype=np.int64)
    group_starts, chunk_rows, chunk_base = chunk_layout(nchunks, gw)
    n = np.arange(N, dtype=np.int64)
    c = n // SHARD
    r = n % SHARD
    pos = np.zeros(N, np.int64)
    for j, gs in enumerate(group_starts):
        lo = gs * gw
        hi = lo + chunk_rows[j]
        m = (r >= lo) & (r < hi)
        pos[m] = chunk_base[j] + c[m] * chunk_rows[j] + (r[m] - lo)
    return pos


def prep_graph(edge_index, ipc=IDX_PER_CALL, gw=GW, nchunks=None):
    if nchunks is None:
        nchunks = BUILD_KW.get("nchunks", 1)
    n_groups = math.ceil(SHARD / gw)
    src = edge_index[0].astype(np.int64)
    dst = edge_index[1].astype(np.int64)
    deg = (np.bincount(dst, minlength=N) + 1).astype(np.float64)  # +1 self
    dinv = 1.0 / np.sqrt(deg)
    norm = (dinv[src] * dinv[dst]).astype(np.float32)
    pos = node_pos(nchunks, gw)
    src = pos[src]              # gather by table position, not node id

    core = dst // SHARD
    gloc = (dst % SHARD) // gw
    half = (src >= HALF).astype(np.int64)
    cell = (core * n_groups + gloc) * 2 + half

    counts = np.bincount(cell, minlength=N_CORES * n_groups * 2)
    counts = counts.reshape(N_CORES, n_groups, 2)
    B = np.ceil(counts / BLK).astype(np.int64).max(axis=0)   # [N_GROUPS, 2]

    # per-half streams; cell (g,h) occupies B[g,h]*BLK slots of stream h
    stream_blocks = [B[:, h].sum() for h in (0, 1)]
    ncalls = [math.ceil(sb * BLK / ipc) for sb in stream_blocks]
    stream_slots = [nc_ * ipc for nc_ in ncalls]
    cell_base = np.zeros((n_groups, 2), np.int64)           # slot base within stream h
    for h in (0, 1):
        cell_base[:, h] = np.cumsum(B[:, h] * BLK) - B[:, h] * BLK

    # rank of each edge within its cell; secondary sort by src so the
    # gather's DMA descriptors read ascending addresses (HBM row-buffer
    # locality)
    order = np.lexsort((src, cell))
    cell_sorted = cell[order]
    starts = np.searchsorted(cell_sorted, np.arange(N_CORES * n_groups * 2))
    rank = np.arange(cell.shape[0]) - starts[cell_sorted]
    # slot within the edge's (core, stream-h): cell_base + rank
    g_s = gloc[order]
    h_s = half[order]
    c_s = core[order]
    slot = cell_base[g_s, h_s] + rank

    idx16 = (src[order] - h_s * HALF).astype(np.int16)
    dstloc = ((dst[order] % SHARD) % gw).astype(np.float32)
    normv = norm[order].astype(np.float32)

    # self-loop diag norms: [128, n_groups*2] per core (col = 2*g + t)
    dinv2 = (dinv * dinv).astype(np.float32)
    nself_cols = n_groups * math.ceil(gw / 128)
    per_core = []
    NBs = [sl // BLK for sl in stream_slots]
    for c in range(N_CORES):
        m = c_s == c
        data = {}
        for h in (0, 1):
            mh = m & (h_s == h)
            idx_flat = np.zeros(stream_slots[h], np.int16)
            dl_flat = np.zeros(stream_slots[h], np.float32)
            nm_flat = np.zeros(stream_slots[h], np.float32)
            s = slot[mh]
            idx_flat[s] = idx16[mh]
            dl_flat[s] = dstloc[mh]
            nm_flat[s] = normv[mh]
            data[f"idx{h}"] = _wrap_idx(idx_flat)
            data[f"dl{h}"] = dl_flat.reshape(NBs[h], BLK).T.copy()   # [128, NB_h]
            data[f"nm{h}"] = nm_flat.reshape(NBs[h], BLK).T.copy()
        nms = np.zeros((128, nself_cols), np.float32)
        for g in range(n_groups):
            for t in range(math.ceil(gw / 128)):
                base = c * SHARD + g * gw + t * 128
                nrows = min(128, max(0, SHARD - (g * gw + t * 128)))
                if nrows > 0:
                    nms[:nrows, 2 * g + t] = dinv2[base:base + nrows]
        data["nmself"] = nms
        per_core.append(data)
    return B, ncalls, NBs, per_core


# ---------------------------------------------------------------- bass kernel

def build_nc(B, ncalls, NBs, ablate=(), reps=1, nq=4, sp=False,
             ipc=IDX_PER_CALL, gbufs=8, gw=GW, deep=True, scratch=49152,
             nchunks=1):
    n_groups = math.ceil(SHARD / gw)
    tpg = math.ceil(gw / 128)          # self blocks per (full) group
    group_starts, chunk_rows, chunk_base = chunk_layout(nchunks, gw)
    last_group_of_chunk = {min(gs + math.ceil(n_groups / nchunks), n_groups) - 1: j
                           for j, gs in enumerate(group_starts)}
    """ablate: subset of {"gather", "onehot", "matmul", "collective", "dense"}
    — drop that phase (wrong results, used for perf bisection only).
    scratch: SWDGE descriptor carveout bytes/partition; ring capacity per
    queue is scratch//16 descs — must exceed ipc for gen/transfer overlap."""
    nc = bacc.Bacc("TRN2", target_bir_lowering=False, debug=False,
                   num_devices=N_CORES, num_swdge_queues=nq,
                   dynamic_dma_scratch_size=scratch)

    x_tab = nc.dram_tensor("x_tab", [N, D], F16, kind="ExternalInput")
    xshard_in = nc.dram_tensor("xshard", [SHARD_PAD, D], F16,
                               kind="ExternalInput")
    idx_in = [nc.dram_tensor(f"idx{h}", [128, ncalls[h] * ipc // 16], I16,
                             kind="ExternalInput") for h in (0, 1)]
    dl_in = [nc.dram_tensor(f"dl{h}", [128, NBs[h]], F32, kind="ExternalInput")
             for h in (0, 1)]
    nm_in = [nc.dram_tensor(f"nm{h}", [128, NBs[h]], F32, kind="ExternalInput")
             for h in (0, 1)]
    nmself_in = nc.dram_tensor("nmself", [128, n_groups * tpg], F32,
                               kind="ExternalInput")
    dlself_in = nc.dram_tensor("dlself", [128, tpg], F32, kind="ExternalInput")
    iota_in = nc.dram_tensor("iota", [128, gw], F16, kind="ExternalInput")
    id16_in = nc.dram_tensor("id16", [128, 128], F16, kind="ExternalInput")
    id32_in = nc.dram_tensor("id32", [128, 128], F32, kind="ExternalInput")
    w_in = [nc.dram_tensor(f"w{l}t", [D, D], F16, kind="ExternalInput")
            for l in range(3)]
    b_in = [nc.dram_tensor(f"b{l}", [128, 1], F32, kind="ExternalInput")
            for l in range(3)]
    y_out = nc.dram_tensor("y", [SHARD, D], F32, kind="ExternalOutput")

    zshard = [nc.dram_tensor(f"z{l}s", [SHARD_PAD, D], F16) for l in range(2)]
    zfull = [nc.dram_tensor(f"z{l}f", [N, D], F16, addr_space="Shared")
             for l in range(2)]

    with tile.TileContext(nc) as tc:
        with tc.tile_pool(name="const", bufs=1) as cpool, \
             tc.tile_pool(name="gp", bufs=2 * gbufs) as g_pool, \
             tc.tile_pool(name="sl", bufs=8) as sl_pool, \
             tc.tile_pool(name="s", bufs=(16 if deep else 4)) as s_pool, \
             tc.tile_pool(name="a", bufs=(4 if deep else 2)) as a_pool, \
             tc.tile_pool(name="z", bufs=(4 if deep else 2)) as z_pool, \
             tc.tile_pool(name="t", bufs=4) as t_pool, \
             tc.tile_pool(name="psa", bufs=(4 if deep else 2), space="PSUM") as psa_pool, \
             tc.tile_pool(name="pso", bufs=2, space="PSUM") as pso_pool, \
             tc.tile_pool(name="pst", bufs=2, space="PSUM") as pst_pool:

            # ---- constants
            idx_t, dl_t, nm_t = [], [], []
            for h in (0, 1):
                it = cpool.tile([128, ncalls[h] * ipc // 16], I16, tag=f"idx{h}")
                nc.sync.dma_start(out=it[:], in_=idx_in[h][:, :])
                idx_t.append(it)
                dt_ = cpool.tile([128, NBs[h]], F32, tag=f"dl{h}")
                nc.sync.dma_start(out=dt_[:], in_=dl_in[h][:, :])
                dl_t.append(dt_)
                nt = cpool.tile([128, NBs[h]], F32, tag=f"nm{h}")
                nc.sync.dma_start(out=nt[:], in_=nm_in[h][:, :])
                nm_t.append(nt)
            nmself_t = cpool.tile([128, n_groups * tpg], F32, tag="nmself")
            nc.sync.dma_start(out=nmself_t[:], in_=nmself_in[:, :])
            dlself_t = cpool.tile([128, tpg], F32, tag="dlself")
            nc.sync.dma_start(out=dlself_t[:], in_=dlself_in[:, :])
            iota_t = cpool.tile([128, gw], F16, tag="iota")
            nc.sync.dma_start(out=iota_t[:], in_=iota_in[:, :])
            id16_t = cpool.tile([128, 128], F16, tag="id16")
            nc.sync.dma_start(out=id16_t[:], in_=id16_in[:, :])
            id32_t = cpool.tile([128, 128], F32, tag="id32")
            nc.sync.dma_start(out=id32_t[:], in_=id32_in[:, :])
            w_t, b_t = [], []
            for l in range(3):
                wt = cpool.tile([D, D], F16, tag=f"w{l}")
                nc.sync.dma_start(out=wt[:], in_=w_in[l][:, :])
                w_t.append(wt)
                bt = cpool.tile([128, 1], F32, tag=f"b{l}")
                nc.sync.dma_start(out=bt[:], in_=b_in[l][:, :])
                b_t.append(bt)

            # zero the zshard pad rows once (self-loop blocks over-read them;
            # nm=0 kills the contribution but the data must be finite)
            zpad = cpool.tile([128, D], F16, tag="zpad")
            nc.vector.memset(zpad[:], 0.0)
            for l in range(2):
                nc.sync.dma_start(out=zshard[l][SHARD:SHARD_PAD, :],
                                  in_=zpad[:SHARD_PAD - SHARD, :])

            s_zero = None
            if "onehot" in ablate:
                s_zero = cpool.tile([128, gw], F16, tag="szero")
                nc.vector.memset(s_zero[:], 0.0)

            # block -> stream slot base for each (g, h)
            cell_base_blk = np.zeros((n_groups, 2), np.int64)
            for h in (0, 1):
                cell_base_blk[:, h] = np.cumsum(B[:, h]) - B[:, h]

            for rep_layer in range(3 * reps):
                layer = rep_layer % 3
                z_tab = [x_tab, zfull[0], zfull[1]][layer]
                z_self = [xshard_in, zshard[0], zshard[1]][layer]
                tabs = [z_tab[0:HALF, :], z_tab[HALF:N, :]]
                g_tiles = [{}, {}]   # per stream: call -> tile

                def get_block(h, blk_i):
                    call = (blk_i * BLK) // ipc
                    j = blk_i - call * (ipc // BLK)
                    if call not in g_tiles[h]:
                        gt = g_pool.tile([128, ipc // BLK, D], F16,
                                         tag="g")
                        if "gather" not in ablate:
                            nc.gpsimd.dma_gather(
                                out_ap=gt[:],
                                in_ap=tabs[h],
                                idxs_ap=idx_t[h][:, call * (ipc // 16):
                                                 (call + 1) * (ipc // 16)],
                                num_idxs=ipc,
                                num_idxs_reg=ipc,
                                elem_size=D,
                                queue_num=(h + 2 * call) % nq,
                                single_packet=sp,
                            )
                        else:
                            nc.vector.memset(gt[:, 0, :], 0.25)
                        g_tiles[h][call] = gt
                    return g_tiles[h][call][:, j, :]

                for g in range(n_groups):
                    gw_act = min(gw, SHARD - g * gw)
                    nself = math.ceil(gw_act / 128)
                    nblk = int(B[g, 0] + B[g, 1]) + nself
                    psA = psa_pool.tile([128, gw], F32, tag="psa")
                    bi = 0
                    # self-loop blocks first (sequential dma, warms PE)
                    for t in range(nself):
                        base = g * gw + t * 128
                        st = sl_pool.tile([128, 128], F16, tag="slf")
                        if "gather" not in ablate:
                            nc.sync.dma_start(
                                out=st[:],
                                in_=z_self[base:base + 128, :],
                            )
                        else:
                            nc.vector.memset(st[:, 0:128], 0.25)
                        if "onehot" not in ablate:
                            sT = s_pool.tile([128, gw], F16, tag="s")
                            nc.vector.tensor_scalar(
                                out=sT[:], in0=iota_t[:],
                                scalar1=dlself_t[:, t:t + 1],
                                scalar2=nmself_t[:, tpg * g + t:tpg * g + t + 1],
                                op0=mybir.AluOpType.is_equal,
                                op1=mybir.AluOpType.mult,
                            )
                        else:
                            sT = s_zero
                        stop_now = (bi == nblk - 1)
                        if "matmul" not in ablate:
                            nc.tensor.matmul(
                                out=psA[:], lhsT=st[:], rhs=sT[:],
                                start=(bi == 0), stop=stop_now,
                            )
                        elif bi == 0:
                            nc.tensor.matmul(out=psA[:], lhsT=st[:], rhs=sT[:],
                                             start=True, stop=True)
                        bi += 1
                    for h in (0, 1):
                        for i in range(int(B[g, h])):
                            blk_i = int(cell_base_blk[g, h] + i)
                            gblk = get_block(h, blk_i)
                            if "onehot" not in ablate:
                                sT = s_pool.tile([128, gw], F16, tag="s")
                                nc.vector.tensor_scalar(
                                    out=sT[:], in0=iota_t[:],
                                    scalar1=dl_t[h][:, blk_i:blk_i + 1],
                                    scalar2=nm_t[h][:, blk_i:blk_i + 1],
                                    op0=mybir.AluOpType.is_equal,
                                    op1=mybir.AluOpType.mult,
                                )
                            else:
                                sT = s_zero
                            if "matmul" not in ablate:
                                nc.tensor.matmul(
                                    out=psA[:], lhsT=gblk, rhs=sT[:],
                                    start=False, stop=(bi == nblk - 1),
                                )
                            bi += 1

                    aT = a_pool.tile([128, gw], F16, tag="a")
                    nc.scalar.activation(out=aT[:], in_=psA[:],
                                         func=mybir.ActivationFunctionType.Copy)
                    psO = pso_pool.tile([128, gw], F32, tag="pso")
                    nc.tensor.matmul(out=psO[:], lhsT=w_t[layer][:], rhs=aT[:],
                                     start=True, stop=True)

                    if layer < 2:
                        zT = z_pool.tile([128, gw], F16, tag="z16")
                        nc.scalar.activation(out=zT[:], in_=psO[:],
                                             func=mybir.ActivationFunctionType.Relu,
                                             bias=b_t[layer][:])
                        ident = id16_t
                        odt = F16
                        dest = zshard[layer]
                    else:
                        zT = z_pool.tile([128, gw], F32, tag="z32")
                        nc.scalar.activation(out=zT[:], in_=psO[:],
                                             func=mybir.ActivationFunctionType.Identity,
                                             bias=b_t[layer][:])
                        ident = id32_t
                        odt = F32
                        dest = y_out

                    for t in range(math.ceil(gw_act / 128)):
                        width = min(128, gw_act - t * 128)
                        psT = pst_pool.tile([128, 128], odt, tag="pst")
                        nc.tensor.transpose(out=psT[:], in_=zT[:, t * 128:(t + 1) * 128],
                                            identity=ident[:])
                        ts_ = t_pool.tile([128, 128], odt, tag="t")
                        nc.scalar.activation(out=ts_[:], in_=psT[:],
                                             func=mybir.ActivationFunctionType.Copy)
                        base = g * gw + t * 128
                        nc.sync.dma_start(out=dest[base:base + width, :],
                                          in_=ts_[:width, :])

                    if layer < 2 and g in last_group_of_chunk:
                        j = last_group_of_chunk[g]
                        r0 = group_starts[j] * gw
                        rj = chunk_rows[j]
                        bj = chunk_base[j]
                        if "collective" in ablate:
                            for c in range(N_CORES):
                                nc.sync.dma_start(
                                    out=zfull[layer][bj + c * rj:
                                                     bj + (c + 1) * rj, :],
                                    in_=zshard[layer][r0:r0 + rj, :])
                        else:
                            nc.gpsimd.collective_compute(
                                "AllGather", mybir.AluOpType.bypass,
                                replica_groups=[list(range(N_CORES))],
                                ins=[zshard[layer][r0:r0 + rj, :].opt()],
                                outs=[zfull[layer][bj:bj + N_CORES * rj, :].opt()],
                            )

    nc.compile()
    return nc


class _MakespanFilter(logging.Filter):
    """Captures the Tile scheduling sim's predicted makespan."""

    def __init__(self):
        super().__init__()
        self.times = []

    def filter(self, record):
        m = re.search(r"Simulation completed at time (\d+)", record.getMessage())
        if m:
            self.times.append(int(m.group(1)))
        return True


def build_with_makespan(*args, **kwargs):
    lg = logging.getLogger("concourse")
    old_level = lg.level
    f = _MakespanFilter()
    lg.addFilter(f)
    lg.setLevel(logging.DEBUG)
    try:
        nc = build_nc(*args, **kwargs)
    finally:
        lg.removeFilter(f)
        lg.setLevel(old_level)
    makespan = max(f.times) if f.times else None
    return nc, makespan


# ---------------------------------------------------------------- runner

class SpmdRunner:
    """Persistent jitted SPMD executor (axon/PJRT path, jit built once)."""

    def __init__(self, nc, n_cores):
        import jax
        from jax.sharding import Mesh, PartitionSpec
        from jax.experimental.shard_map import shard_map
        from concourse.bass2jax import (_bass_exec_p, install_neuronx_cc_hook,
                                        partition_id_tensor)
        install_neuronx_cc_hook()
        self.jax = jax
        self.nc = nc
        self.n_cores = n_cores
        partition_name = nc.partition_id_tensor.name if nc.partition_id_tensor else None
        in_names, out_names, out_avals, zero_outs = [], [], [], []
        for alloc in nc.m.functions[0].allocations:
            if not isinstance(alloc, mybir.MemoryLocationSet):
                continue
            name = alloc.memorylocations[0].name
            if alloc.kind == "ExternalInput":
                if name != partition_name:
                    in_names.append(name)
            elif alloc.kind == "ExternalOutput":
                shape = tuple(alloc.tensor_shape)
                dtype = mybir.dt.np(alloc.dtype)
                out_names.append(name)
                out_avals.append(jax.core.ShapedArray(shape, dtype))
                zero_outs.append(np.zeros(shape, dtype))
        self.in_names, self.out_names = in_names, out_names
        self.out_avals, self.zero_outs = out_avals, zero_outs
        n_params, n_outs = len(in_names), len(out_avals)
        all_in = list(in_names) + list(out_names)
        if partition_name is not None:
            all_in.append(partition_name)

        def _body(*args):
            operands = list(args)
            if partition_name is not None:
                operands.append(partition_id_tensor())
            outs = _bass_exec_p.bind(
                *operands, out_avals=tuple(out_avals), in_names=tuple(all_in),
                out_names=tuple(out_names), lowering_input_output_aliases=(),
                sim_require_finite=True, sim_require_nnan=True, nc=nc)
            return tuple(outs)

        devices = jax.devices()[:n_cores]
        mesh = Mesh(np.asarray(devices), ("core",))
        from jax.sharding import NamedSharding
        from jax.sharding import PartitionSpec as P
        self._sharding = NamedSharding(mesh, P("core"))
        self._fn = jax.jit(
            shard_map(_body, mesh=mesh,
                      in_specs=(P("core"),) * (n_params + n_outs),
                      out_specs=(P("core"),) * n_outs, check_rep=False),
            keep_unused=True)
        self._staged = None

    def stage_inputs(self, in_maps):
        n = self.n_cores
        concat = [np.concatenate([np.asarray(in_maps[c][nm]) for c in range(n)], axis=0)
                  for nm in self.in_names]
        concat += [np.zeros((n * z.shape[0], *z.shape[1:]), z.dtype)
                   for z in self.zero_outs]
        self._staged = [self.jax.device_put(a, self._sharding) for a in concat]

    def run(self):
        outs = self._fn(*self._staged)
        self.jax.block_until_ready(outs)
        return outs

    def results(self, outs):
        res = []
        for c in range(self.n_cores):
            m = {}
            for i, nm in enumerate(self.out_names):
                full = np.asarray(outs[i])
                m[nm] = full.reshape(self.n_cores, *self.out_avals[i].shape)[c]
            res.append(m)
        return res


_CACHE = {}

# default build configuration (tuned)
BUILD_KW = dict(sp=True, nq=4, gbufs=40, scratch=16384, nchunks=3)


def _get_built(B_key, B, ncalls, NBs, reps=1):
    key = (B_key, reps)
    if key not in _CACHE:
        nc, makespan = build_with_makespan(B, ncalls, NBs, reps=reps,
                                           **BUILD_KW)
        if makespan:
            print(f"[kernel] predicted makespan: {makespan} ns")
        _CACHE[key] = (nc, SpmdRunner(nc, N_CORES))
    return _CACHE[key]


def build_in_maps(inputs, per_core, nchunks=None):
    """Per-core input dicts from the full-problem input dict."""
    if nchunks is None:
        nchunks = BUILD_KW.get("nchunks", 1)
    x16n = np.asarray(inputs["x"]).astype(np.float16)
    x16 = np.empty_like(x16n)
    x16[node_pos(nchunks)] = x16n          # chunked table layout
    iota = np.tile(np.arange(GW, dtype=np.float16), (128, 1))
    ident = np.eye(128)
    tpg = math.ceil(GW / 128)
    dlself = np.zeros((128, tpg), np.float32)
    for t in range(tpg):
        dlself[:, t] = t * 128 + np.arange(128)
    xsh = np.zeros((N_CORES, SHARD_PAD, D), np.float16)
    xsh[:, 0:SHARD] = x16n.reshape(N_CORES, SHARD, D)
    common = {
        "x_tab": x16,
        "iota": iota,
        "dlself": dlself,
        "id16": ident.astype(np.float16),
        "id32": ident.astype(np.float32),
        "w0t": np.asarray(inputs["W1"]).T.astype(np.float16),
        "w1t": np.asarray(inputs["W2"]).T.astype(np.float16),
        "w2t": np.asarray(inputs["W3"]).T.astype(np.float16),
        "b0": np.asarray(inputs["b1"]).reshape(128, 1).astype(np.float32),
        "b1": np.asarray(inputs["b2"]).reshape(128, 1).astype(np.float32),
        "b2": np.asarray(inputs["b3"]).reshape(128, 1).astype(np.float32),
    }
    return [{**common, "xshard": xsh[c], **per_core[c]} for c in range(N_CORES)]


def kernel(x, edge_index, W1, b1, W2, b2, W3, b3):
    x = np.asarray(x)
    edge_index = np.asarray(edge_index)
    nchunks = BUILD_KW.get("nchunks", 1)
    B, ncalls, NBs, per_core = prep_graph(edge_index, nchunks=nchunks)
    B_key = (tuple(B.flatten().tolist()), tuple(ncalls))
    nc, runner = _get_built(B_key, B, ncalls, NBs)
    in_maps = build_in_maps(
        {"x": x, "W1": W1, "b1": b1, "W2": W2, "b2": b2, "W3": W3, "b3": b3},
        per_core, nchunks=nchunks)
    runner.stage_inputs(in_maps)
    outs = runner.run()
    res = runner.results(outs)
    return np.concatenate([res[c]["y"] for c in range(N_CORES)], axis=0)


# revision 34
# speedup vs baseline: 1.2411x; 1.2411x over previous
"""3-layer GCN (message passing) on 8 Trainium2 NeuronCores.

Strategy (graph/data parallel, per sharding hint):
  - Nodes sharded by destination across 8 cores (6250 dst rows each);
    edges bucketed by dst owner on the host; weights replicated.
  - Per layer:  out = Ahat @ (z @ W^T) + b  ==  (Ahat @ z) @ W^T + b
    where Ahat = D^-1/2 (A+I) D^-1/2.  Each core computes its dst shard:
      1. real edges: gather z[src] rows (fp16) for its edges via
         dma_gather (4 SWDGE queues) from a full local fp16 replica of z,
      2. self-loops: sequential dma_start of the core's own shard rows
         (no gather indices needed; one-hot uses dinv^2 diag columns),
      3. scatter-add into 256-dst PSUM groups via one-hot matmul
         (one-hot built on DVE: (iota == dst_local) * norm),
      4. dense W^T matmul (feature-major), bias+ReLU on ACT,
      5. transpose to node-major and store the shard,
      6. AllGather the fp16 shards -> full z for the next layer.
  - PSUM->SBUF copies and bias adds run on the ACT engine so DVE does
    only the one-hot builds.
  - Graph prep (degrees, norms, edge bucketing/padding) is host-side.
"""
import logging
import math
import re

import numpy as np

import concourse.bass as bass
import concourse.tile as tile
from concourse import bacc, mybir

N = 50000
E = 600000
D = 128
N_CORES = 8
SHARD = N // N_CORES          # 6250
GW = 256                      # dst-group width (psum group)
N_GROUPS = math.ceil(SHARD / GW)   # 25 (24*256 + 106)
HALF = N // 2                 # gather-table halves (int16 index limit)
SHARD_PAD = 6272              # self-loop block overread pad (24*256+128)
IDX_PER_CALL = 1024
BLK = 128
F16 = mybir.dt.float16
F32 = mybir.dt.float32
I16 = mybir.dt.int16


# ---------------------------------------------------------------- host prep

def _wrap_idx(flat):
    """dma_gather index layout: [128, S/16] int16, idx i at [i%16, i//16],
    replicated across the 8 gpsimd 16-partition groups."""
    S = flat.shape[0]
    arr = np.zeros((128, S // 16), np.int16)
    w = flat.reshape(S // 16, 16).T          # [16, S/16]
    for grp in range(8):
        arr[grp * 16:(grp + 1) * 16, :] = w
    return arr


def chunk_layout(nchunks, gw=GW):
    """Chunked zfull layout: chunk j holds [core0 rows, core1 rows, ...] for
    a contiguous range of dst groups, so each chunk's AllGather is one
    contiguous in/out slice and can fire as soon as its groups are stored.
    Returns (group_starts, chunk_rows, chunk_base) with per-chunk group
    ranges, per-core row counts, and zfull base offsets."""
    n_groups = math.ceil(SHARD / gw)
    gpc = math.ceil(n_groups / nchunks)
    group_starts = list(range(0, n_groups, gpc))
    chunk_rows, chunk_base = [], []
    base = 0
    for j, gs in enumerate(group_starts):
        ge = min(gs + gpc, n_groups)
        rows = min(ge * gw, SHARD) - gs * gw
        chunk_rows.append(rows)
        chunk_base.append(base)
        base += N_CORES * rows
    return group_starts, chunk_rows, chunk_base


def node_pos(nchunks, gw=GW):
    """Position of each node in the chunked zfull layout ([N] int64)."""
    if nchunks <= 1:
        return np.arange(N, dtype=np.int64)
    group_starts, chunk_rows, chunk_base = chunk_layout(nchunks, gw)
    n = np.arange(N, dtype=np.int64)
    c = n // SHARD
    r = n % SHARD
    pos = np.zeros(N, np.int64)
    for j, gs in enumerate(group_starts):
        lo = gs * gw
        hi = lo + chunk_rows[j]
        m = (r >= lo) & (r < hi)
        pos[m] = chunk_base[j] + c[m] * chunk_rows[j] + (r[m] - lo)
    return pos


def prep_graph(edge_index, ipc=IDX_PER_CALL, gw=GW, nchunks=None):
    if nchunks is None:
        nchunks = BUILD_KW.get("nchunks", 1)
    n_groups = math.ceil(SHARD / gw)
    src = edge_index[0].astype(np.int64)
    dst = edge_index[1].astype(np.int64)
    deg = (np.bincount(dst, minlength=N) + 1).astype(np.float64)  # +1 self
    dinv = 1.0 / np.sqrt(deg)
    norm = (dinv[src] * dinv[dst]).astype(np.float32)
    pos = node_pos(nchunks, gw)
    src = pos[src]              # gather by table position, not node id

    core = dst // SHARD
    gloc = (dst % SHARD) // gw
    half = (src >= HALF).astype(np.int64)
    cell = (core * n_groups + gloc) * 2 + half

    counts = np.bincount(cell, minlength=N_CORES * n_groups * 2)
    counts = counts.reshape(N_CORES, n_groups, 2)
    B = np.ceil(counts / BLK).astype(np.int64).max(axis=0)   # [N_GROUPS, 2]

    # per-half streams; cell (g,h) occupies B[g,h]*BLK slots of stream h
    stream_blocks = [B[:, h].sum() for h in (0, 1)]
    ncalls = [math.ceil(sb * BLK / ipc) for sb in stream_blocks]
    stream_slots = [nc_ * ipc for nc_ in ncalls]
    cell_base = np.zeros((n_groups, 2), np.int64)           # slot base within stream h
    for h in (0, 1):
        cell_base[:, h] = np.cumsum(B[:, h] * BLK) - B[:, h] * BLK

    # rank of each edge within its cell; secondary sort by src so the
    # gather's DMA descriptors read ascending addresses (HBM row-buffer
    # locality)
    order = np.lexsort((src, cell))
    cell_sorted = cell[order]
    starts = np.searchsorted(cell_sorted, np.arange(N_CORES * n_groups * 2))
    rank = np.arange(cell.shape[0]) - starts[cell_sorted]
    # slot within the edge's (core, stream-h): cell_base + rank
    g_s = gloc[order]
    h_s = half[order]
    c_s = core[order]
    slot = cell_base[g_s, h_s] + rank

    idx16 = (src[order] - h_s * HALF).astype(np.int16)
    dstloc = ((dst[order] % SHARD) % gw).astype(np.float32)
    normv = norm[order].astype(np.float32)

    # self-loop diag norms: [128, n_groups*2] per core (col = 2*g + t)
    dinv2 = (dinv * dinv).astype(np.float32)
    nself_cols = n_groups * math.ceil(gw / 128)
    per_core = []
    NBs = [sl // BLK for sl in stream_slots]
    for c in range(N_CORES):
        m = c_s == c
        data = {}
        for h in (0, 1):
            mh = m & (h_s == h)
            idx_flat = np.zeros(stream_slots[h], np.int16)
            dl_flat = np.zeros(stream_slots[h], np.float32)
            nm_flat = np.zeros(stream_slots[h], np.float32)
            s = slot[mh]
            idx_flat[s] = idx16[mh]
            dl_flat[s] = dstloc[mh]
            nm_flat[s] = normv[mh]
            data[f"idx{h}"] = _wrap_idx(idx_flat)
            data[f"dl{h}"] = dl_flat.reshape(NBs[h], BLK).T.copy()   # [128, NB_h]
            data[f"nm{h}"] = nm_flat.reshape(NBs[h], BLK).T.copy()
        nms = np.zeros((128, nself_cols), np.float32)
        for g in range(n_groups):
            for t in range(math.ceil(gw / 128)):
                base = c * SHARD + g * gw + t * 128
                nrows = min(128, max(0, SHARD - (g * gw + t * 128)))
                if nrows > 0:
                    nms[:nrows, 2 * g + t] = dinv2[base:base + nrows]
        data["nmself"] = nms
        per_core.append(data)
    return B, ncalls, NBs, per_core


# ---------------------------------------------------------------- bass kernel

def build_nc(B, ncalls, NBs, ablate=(), reps=1, nq=4, sp=False,
             ipc=IDX_PER_CALL, gbufs=8, gw=GW, deep=True, scratch=49152,
             nchunks=1):
    n_groups = math.ceil(SHARD / gw)
    tpg = math.ceil(gw / 128)          # self blocks per (full) group
    group_starts, chunk_rows, chunk_base = chunk_layout(nchunks, gw)
    last_group_of_chunk = {min(gs + math.ceil(n_groups / nchunks), n_groups) - 1: j
                           for j, gs in enumerate(group_starts)}
    """ablate: subset of {"gather", "onehot", "matmul", "collective", "dense"}
    — drop that phase (wrong results, used for perf bisection only).
    scratch: SWDGE descriptor carveout bytes/partition; ring capacity per
    queue is scratch//16 descs — must exceed ipc for gen/transfer overlap."""
    nc = bacc.Bacc("TRN2", target_bir_lowering=False, debug=False,
                   num_devices=N_CORES, num_swdge_queues=nq,
                   dynamic_dma_scratch_size=scratch)

    x_tab = nc.dram_tensor("x_tab", [N, D], F16, kind="ExternalInput")
    xshard_in = nc.dram_tensor("xshard", [SHARD_PAD, D], F16,
                               kind="ExternalInput")
    idx_in = [nc.dram_tensor(f"idx{h}", [128, ncalls[h] * ipc // 16], I16,
                             kind="ExternalInput") for h in (0, 1)]
    dl_in = [nc.dram_tensor(f"dl{h}", [128, NBs[h]], F32, kind="ExternalInput")
             for h in (0, 1)]
    nm_in = [nc.dram_tensor(f"nm{h}", [128, NBs[h]], F32, kind="ExternalInput")
             for h in (0, 1)]
    nmself_in = nc.dram_tensor("nmself", [128, n_groups * tpg], F32,
                               kind="ExternalInput")
    dlself_in = nc.dram_tensor("dlself", [128, tpg], F32, kind="ExternalInput")
    iota_in = nc.dram_tensor("iota", [128, gw], F16, kind="ExternalInput")
    id16_in = nc.dram_tensor("id16", [128, 128], F16, kind="ExternalInput")
    id32_in = nc.dram_tensor("id32", [128, 128], F32, kind="ExternalInput")
    w_in = [nc.dram_tensor(f"w{l}t", [D, D], F16, kind="ExternalInput")
            for l in range(3)]
    b_in = [nc.dram_tensor(f"b{l}", [128, 1], F32, kind="ExternalInput")
            for l in range(3)]
    y_out = nc.dram_tensor("y", [SHARD, D], F32, kind="ExternalOutput")

    zshard = [nc.dram_tensor(f"z{l}s", [SHARD_PAD, D], F16) for l in range(2)]
    zfull = [nc.dram_tensor(f"z{l}f", [N, D], F16, addr_space="Shared")
             for l in range(2)]

    with tile.TileContext(nc) as tc:
        with tc.tile_pool(name="const", bufs=1) as cpool, \
             tc.tile_pool(name="gp", bufs=2 * gbufs) as g_pool, \
             tc.tile_pool(name="sl", bufs=8) as sl_pool, \
             tc.tile_pool(name="s", bufs=(24 if deep else 4)) as s_pool, \
             tc.tile_pool(name="a", bufs=(4 if deep else 2)) as a_pool, \
             tc.tile_pool(name="z", bufs=(4 if deep else 2)) as z_pool, \
             tc.tile_pool(name="t", bufs=4) as t_pool, \
             tc.tile_pool(name="psa", bufs=(4 if deep else 2), space="PSUM") as psa_pool, \
             tc.tile_pool(name="pso", bufs=2, space="PSUM") as pso_pool, \
             tc.tile_pool(name="pst", bufs=2, space="PSUM") as pst_pool:

            # ---- constants
            idx_t, dl_t, nm_t = [], [], []
            for h in (0, 1):
                it = cpool.tile([128, ncalls[h] * ipc // 16], I16, tag=f"idx{h}")
                nc.sync.dma_start(out=it[:], in_=idx_in[h][:, :])
                idx_t.append(it)
                dt_ = cpool.tile([128, NBs[h]], F32, tag=f"dl{h}")
                nc.sync.dma_start(out=dt_[:], in_=dl_in[h][:, :])
                dl_t.append(dt_)
                nt = cpool.tile([128, NBs[h]], F32, tag=f"nm{h}")
                nc.sync.dma_start(out=nt[:], in_=nm_in[h][:, :])
                nm_t.append(nt)
            nmself_t = cpool.tile([128, n_groups * tpg], F32, tag="nmself")
            nc.sync.dma_start(out=nmself_t[:], in_=nmself_in[:, :])
            dlself_t = cpool.tile([128, tpg], F32, tag="dlself")
            nc.sync.dma_start(out=dlself_t[:], in_=dlself_in[:, :])
            iota_t = cpool.tile([128, gw], F16, tag="iota")
            nc.sync.dma_start(out=iota_t[:], in_=iota_in[:, :])
            id16_t = cpool.tile([128, 128], F16, tag="id16")
            nc.sync.dma_start(out=id16_t[:], in_=id16_in[:, :])
            id32_t = cpool.tile([128, 128], F32, tag="id32")
            nc.sync.dma_start(out=id32_t[:], in_=id32_in[:, :])
            w_t, b_t = [], []
            for l in range(3):
                wt = cpool.tile([D, D], F16, tag=f"w{l}")
                nc.sync.dma_start(out=wt[:], in_=w_in[l][:, :])
                w_t.append(wt)
                bt = cpool.tile([128, 1], F32, tag=f"b{l}")
                nc.sync.dma_start(out=bt[:], in_=b_in[l][:, :])
                b_t.append(bt)

            # zero the zshard pad rows once (self-loop blocks over-read them;
            # nm=0 kills the contribution but the data must be finite)
            zpad = cpool.tile([128, D], F16, tag="zpad")
            nc.vector.memset(zpad[:], 0.0)
            for l in range(2):
                nc.sync.dma_start(out=zshard[l][SHARD:SHARD_PAD, :],
                                  in_=zpad[:SHARD_PAD - SHARD, :])

            s_zero = None
            if "onehot" in ablate:
                s_zero = cpool.tile([128, gw], F16, tag="szero")
                nc.vector.memset(s_zero[:], 0.0)

            # block -> stream slot base for each (g, h)
            cell_base_blk = np.zeros((n_groups, 2), np.int64)
            for h in (0, 1):
                cell_base_blk[:, h] = np.cumsum(B[:, h]) - B[:, h]

            for rep_layer in range(3 * reps):
                layer = rep_layer % 3
                z_tab = [x_tab, zfull[0], zfull[1]][layer]
                z_self = [xshard_in, zshard[0], zshard[1]][layer]
                tabs = [z_tab[0:HALF, :], z_tab[HALF:N, :]]
                g_tiles = [{}, {}]   # per stream: call -> tile

                def get_block(h, blk_i):
                    call = (blk_i * BLK) // ipc
                    j = blk_i - call * (ipc // BLK)
                    if call not in g_tiles[h]:
                        gt = g_pool.tile([128, ipc // BLK, D], F16,
                                         tag="g")
                        if "gather" not in ablate:
                            nc.gpsimd.dma_gather(
                                out_ap=gt[:],
                                in_ap=tabs[h],
                                idxs_ap=idx_t[h][:, call * (ipc // 16):
                                                 (call + 1) * (ipc // 16)],
                                num_idxs=ipc,
                                num_idxs_reg=ipc,
                                elem_size=D,
                                queue_num=(h + 2 * call) % nq,
                                single_packet=sp,
                            )
                        else:
                            nc.vector.memset(gt[:, 0, :], 0.25)
                        g_tiles[h][call] = gt
                    return g_tiles[h][call][:, j, :]

                for g in range(n_groups):
                    gw_act = min(gw, SHARD - g * gw)
                    nself = math.ceil(gw_act / 128)
                    nblk = int(B[g, 0] + B[g, 1]) + nself
                    psA = psa_pool.tile([128, gw], F32, tag="psa")
                    bi = 0
                    # self-loop blocks first (sequential dma, warms PE)
                    for t in range(nself):
                        base = g * gw + t * 128
                        st = sl_pool.tile([128, 128], F16, tag="slf")
                        if "gather" not in ablate:
                            nc.sync.dma_start(
                                out=st[:],
                                in_=z_self[base:base + 128, :],
                            )
                        else:
                            nc.vector.memset(st[:, 0:128], 0.25)
                        if "onehot" not in ablate:
                            sT = s_pool.tile([128, gw], F16, tag="s")
                            nc.vector.tensor_scalar(
                                out=sT[:], in0=iota_t[:],
                                scalar1=dlself_t[:, t:t + 1],
                                scalar2=nmself_t[:, tpg * g + t:tpg * g + t + 1],
                                op0=mybir.AluOpType.is_equal,
                                op1=mybir.AluOpType.mult,
                            )
                        else:
                            sT = s_zero
                        stop_now = (bi == nblk - 1)
                        if "matmul" not in ablate:
                            nc.tensor.matmul(
                                out=psA[:], lhsT=st[:], rhs=sT[:],
                                start=(bi == 0), stop=stop_now,
                            )
                        elif bi == 0:
                            nc.tensor.matmul(out=psA[:], lhsT=st[:], rhs=sT[:],
                                             start=True, stop=True)
                        bi += 1
                    for h in (0, 1):
                        for i in range(int(B[g, h])):
                            blk_i = int(cell_base_blk[g, h] + i)
                            gblk = get_block(h, blk_i)
                            if "onehot" not in ablate:
                                sT = s_pool.tile([128, gw], F16, tag="s")
                                nc.vector.tensor_scalar(
                                    out=sT[:], in0=iota_t[:],
                                    scalar1=dl_t[h][:, blk_i:blk_i + 1],
                                    scalar2=nm_t[h][:, blk_i:blk_i + 1],
                                    op0=mybir.AluOpType.is_equal,
                                    op1=mybir.AluOpType.mult,
                                )
                            else:
                                sT = s_zero
                            if "matmul" not in ablate:
                                nc.tensor.matmul(
                                    out=psA[:], lhsT=gblk, rhs=sT[:],
                                    start=False, stop=(bi == nblk - 1),
                                )
                            bi += 1

                    aT = a_pool.tile([128, gw], F16, tag="a")
                    nc.scalar.activation(out=aT[:], in_=psA[:],
                                         func=mybir.ActivationFunctionType.Copy)
                    psO = pso_pool.tile([128, gw], F32, tag="pso")
                    nc.tensor.matmul(out=psO[:], lhsT=w_t[layer][:], rhs=aT[:],
                                     start=True, stop=True)

                    if layer < 2:
                        zT = z_pool.tile([128, gw], F16, tag="z16")
                        nc.scalar.activation(out=zT[:], in_=psO[:],
                                             func=mybir.ActivationFunctionType.Relu,
                                             bias=b_t[layer][:])
                        ident = id16_t
                        odt = F16
                        dest = zshard[layer]
                    else:
                        zT = z_pool.tile([128, gw], F32, tag="z32")
                        nc.scalar.activation(out=zT[:], in_=psO[:],
                                             func=mybir.ActivationFunctionType.Identity,
                                             bias=b_t[layer][:])
                        ident = id32_t
                        odt = F32
                        dest = y_out

                    for t in range(math.ceil(gw_act / 128)):
                        width = min(128, gw_act - t * 128)
                        psT = pst_pool.tile([128, 128], odt, tag="pst")
                        nc.tensor.transpose(out=psT[:], in_=zT[:, t * 128:(t + 1) * 128],
                                            identity=ident[:])
                        ts_ = t_pool.tile([128, 128], odt, tag="t")
                        nc.scalar.activation(out=ts_[:], in_=psT[:],
                                             func=mybir.ActivationFunctionType.Copy)
                        base = g * gw + t * 128
                        nc.sync.dma_start(out=dest[base:base + width, :],
                                          in_=ts_[:width, :])

                    if layer < 2 and g in last_group_of_chunk:
                        j = last_group_of_chunk[g]
                        r0 = group_starts[j] * gw
                        rj = chunk_rows[j]
                        bj = chunk_base[j]
                        if "collective" in ablate:
                            for c in range(N_CORES):
                                nc.sync.dma_start(
                                    out=zfull[layer][bj + c * rj:
                                                     bj + (c + 1) * rj, :],
                                    in_=zshard[layer][r0:r0 + rj, :])
                        else:
                            nc.gpsimd.collective_compute(
                                "AllGather", mybir.AluOpType.bypass,
                                replica_groups=[list(range(N_CORES))],
                                ins=[zshard[layer][r0:r0 + rj, :].opt()],
                                outs=[zfull[layer][bj:bj + N_CORES * rj, :].opt()],
                            )

    nc.compile()
    return nc


class _MakespanFilter(logging.Filter):
    """Captures the Tile scheduling sim's predicted makespan."""

    def __init__(self):
        super().__init__()
        self.times = []

    def filter(self, record):
        m = re.search(r"Simulation completed at time (\d+)", record.getMessage())
        if m:
            self.times.append(int(m.group(1)))
        return True


def build_with_makespan(*args, **kwargs):
    lg = logging.getLogger("concourse")
    old_level = lg.level
    f = _MakespanFilter()
    lg.addFilter(f)
    lg.setLevel(logging.DEBUG)
    try:
        nc = build_nc(*args, **kwargs)
    finally:
        lg.removeFilter(f)
        lg.setLevel(old_level)
    makespan = max(f.times) if f.times else None
    return nc, makespan


# ---------------------------------------------------------------- runner

class SpmdRunner:
    """Persistent jitted SPMD executor (axon/PJRT path, jit built once)."""

    def __init__(self, nc, n_cores):
        import jax
        from jax.sharding import Mesh, PartitionSpec
        from jax.experimental.shard_map import shard_map
        from concourse.bass2jax import (_bass_exec_p, install_neuronx_cc_hook,
                                        partition_id_tensor)
        install_neuronx_cc_hook()
        self.jax = jax
        self.nc = nc
        self.n_cores = n_cores
        partition_name = nc.partition_id_tensor.name if nc.partition_id_tensor else None
        in_names, out_names, out_avals, zero_outs = [], [], [], []
        for alloc in nc.m.functions[0].allocations:
            if not isinstance(alloc, mybir.MemoryLocationSet):
                continue
            name = alloc.memorylocations[0].name
            if alloc.kind == "ExternalInput":
                if name != partition_name:
                    in_names.append(name)
            elif alloc.kind == "ExternalOutput":
                shape = tuple(alloc.tensor_shape)
                dtype = mybir.dt.np(alloc.dtype)
                out_names.append(name)
                out_avals.append(jax.core.ShapedArray(shape, dtype))
                zero_outs.append(np.zeros(shape, dtype))
        self.in_names, self.out_names = in_names, out_names
        self.out_avals, self.zero_outs = out_avals, zero_outs
        n_params, n_outs = len(in_names), len(out_avals)
        all_in = list(in_names) + list(out_names)
        if partition_name is not None:
            all_in.append(partition_name)

        def _body(*args):
            operands = list(args)
            if partition_name is not None:
                operands.append(partition_id_tensor())
            outs = _bass_exec_p.bind(
                *operands, out_avals=tuple(out_avals), in_names=tuple(all_in),
                out_names=tuple(out_names), lowering_input_output_aliases=(),
                sim_require_finite=True, sim_require_nnan=True, nc=nc)
            return tuple(outs)

        devices = jax.devices()[:n_cores]
        mesh = Mesh(np.asarray(devices), ("core",))
        from jax.sharding import NamedSharding
        from jax.sharding import PartitionSpec as P
        self._sharding = NamedSharding(mesh, P("core"))
        self._fn = jax.jit(
            shard_map(_body, mesh=mesh,
                      in_specs=(P("core"),) * (n_params + n_outs),
                      out_specs=(P("core"),) * n_outs, check_rep=False),
            keep_unused=True)
        self._staged = None

    def stage_inputs(self, in_maps):
        n = self.n_cores
        concat = [np.concatenate([np.asarray(in_maps[c][nm]) for c in range(n)], axis=0)
                  for nm in self.in_names]
        concat += [np.zeros((n * z.shape[0], *z.shape[1:]), z.dtype)
                   for z in self.zero_outs]
        self._staged = [self.jax.device_put(a, self._sharding) for a in concat]

    def run(self):
        outs = self._fn(*self._staged)
        self.jax.block_until_ready(outs)
        return outs

    def results(self, outs):
        res = []
        for c in range(self.n_cores):
            m = {}
            for i, nm in enumerate(self.out_names):
                full = np.asarray(outs[i])
                m[nm] = full.reshape(self.n_cores, *self.out_avals[i].shape)[c]
            res.append(m)
        return res


_CACHE = {}

# default build configuration (tuned)
BUILD_KW = dict(sp=True, nq=4, gbufs=40, scratch=16384, nchunks=3)


def _get_built(B_key, B, ncalls, NBs, reps=1):
    key = (B_key, reps)
    if key not in _CACHE:
        nc, makespan = build_with_makespan(B, ncalls, NBs, reps=reps,
                                           **BUILD_KW)
        if makespan:
            print(f"[kernel] predicted makespan: {makespan} ns")
        _CACHE[key] = (nc, SpmdRunner(nc, N_CORES))
    return _CACHE[key]


def build_in_maps(inputs, per_core, nchunks=None):
    """Per-core input dicts from the full-problem input dict."""
    if nchunks is None:
        nchunks = BUILD_KW.get("nchunks", 1)
    x16n = np.asarray(inputs["x"]).astype(np.float16)
    x16 = np.empty_like(x16n)
    x16[node_pos(nchunks)] = x16n          # chunked table layout
    iota = np.tile(np.arange(GW, dtype=np.float16), (128, 1))
    ident = np.eye(128)
    tpg = math.ceil(GW / 128)
    dlself = np.zeros((128, tpg), np.float32)
    for t in range(tpg):
        dlself[:, t] = t * 128 + np.arange(128)
    xsh = np.zeros((N_CORES, SHARD_PAD, D), np.float16)
    xsh[:, 0:SHARD] = x16n.reshape(N_CORES, SHARD, D)
    common = {
        "x_tab": x16,
        "iota": iota,
        "dlself": dlself,
        "id16": ident.astype(np.float16),
        "id32": ident.astype(np.float32),
        "w0t": np.asarray(inputs["W1"]).T.astype(np.float16),
        "w1t": np.asarray(inputs["W2"]).T.astype(np.float16),
        "w2t": np.asarray(inputs["W3"]).T.astype(np.float16),
        "b0": np.asarray(inputs["b1"]).reshape(128, 1).astype(np.float32),
        "b1": np.asarray(inputs["b2"]).reshape(128, 1).astype(np.float32),
        "b2": np.asarray(inputs["b3"]).reshape(128, 1).astype(np.float32),
    }
    return [{**common, "xshard": xsh[c], **per_core[c]} for c in range(N_CORES)]


def kernel(x, edge_index, W1, b1, W2, b2, W3, b3):
    x = np.asarray(x)
    edge_index = np.asarray(edge_index)
    nchunks = BUILD_KW.get("nchunks", 1)
    B, ncalls, NBs, per_core = prep_graph(edge_index, nchunks=nchunks)
    B_key = (tuple(B.flatten().tolist()), tuple(ncalls))
    nc, runner = _get_built(B_key, B, ncalls, NBs)
    in_maps = build_in_maps(
        {"x": x, "W1": W1, "b1": b1, "W2": W2, "b2": b2, "W3": W3, "b3": b3},
        per_core, nchunks=nchunks)
    runner.stage_inputs(in_maps)
    outs = runner.run()
    res = runner.results(outs)
    return np.concatenate([res[c]["y"] for c in range(N_CORES)], axis=0)


# revision 35
# speedup vs baseline: 1.3273x; 1.0695x over previous
"""3-layer GCN (message passing) on 8 Trainium2 NeuronCores.

Strategy (graph/data parallel, per sharding hint):
  - Nodes sharded by destination across 8 cores (6250 dst rows each);
    edges bucketed by dst owner on the host; weights replicated.
  - Per layer:  out = Ahat @ (z @ W^T) + b  ==  (Ahat @ z) @ W^T + b
    where Ahat = D^-1/2 (A+I) D^-1/2.  Each core computes its dst shard:
      1. real edges: gather z[src] rows (fp16) for its edges via
         dma_gather (4 SWDGE queues) from a full local fp16 replica of z,
      2. self-loops: sequential dma_start of the core's own shard rows
         (no gather indices needed; one-hot uses dinv^2 diag columns),
      3. scatter-add into 256-dst PSUM groups via one-hot matmul
         (one-hot built on DVE: (iota == dst_local) * norm),
      4. dense W^T matmul (feature-major), bias+ReLU on ACT,
      5. transpose to node-major and store the shard,
      6. AllGather the fp16 shards -> full z for the next layer.
  - PSUM->SBUF copies and bias adds run on the ACT engine so DVE does
    only the one-hot builds.
  - Graph prep (degrees, norms, edge bucketing/padding) is host-side.
"""
import logging
import math
import re

import numpy as np

import concourse.bass as bass
import concourse.tile as tile
from concourse import bacc, mybir

N = 50000
E = 600000
D = 128
N_CORES = 8
SHARD = N // N_CORES          # 6250
GW = 256                      # dst-group width (psum group)
N_GROUPS = math.ceil(SHARD / GW)   # 25 (24*256 + 106)
HALF = N // 2                 # gather-table halves (int16 index limit)
SHARD_PAD = 6272              # self-loop block overread pad (24*256+128)
IDX_PER_CALL = 1024
BLK = 128
F16 = mybir.dt.float16
F32 = mybir.dt.float32
I16 = mybir.dt.int16


# ---------------------------------------------------------------- host prep

def _wrap_idx(flat):
    """dma_gather index layout: [128, S/16] int16, idx i at [i%16, i//16],
    replicated across the 8 gpsimd 16-partition groups."""
    S = flat.shape[0]
    arr = np.zeros((128, S // 16), np.int16)
    w = flat.reshape(S // 16, 16).T          # [16, S/16]
    for grp in range(8):
        arr[grp * 16:(grp + 1) * 16, :] = w
    return arr


def chunk_layout(nchunks, gw=GW):
    """Chunked zfull layout: chunk j holds [core0 rows, core1 rows, ...] for
    a contiguous range of dst groups, so each chunk's AllGather is one
    contiguous in/out slice and can fire as soon as its groups are stored.
    Returns (group_starts, chunk_rows, chunk_base) with per-chunk group
    ranges, per-core row counts, and zfull base offsets."""
    n_groups = math.ceil(SHARD / gw)
    gpc = math.ceil(n_groups / nchunks)
    group_starts = list(range(0, n_groups, gpc))
    chunk_rows, chunk_base = [], []
    base = 0
    for j, gs in enumerate(group_starts):
        ge = min(gs + gpc, n_groups)
        rows = min(ge * gw, SHARD) - gs * gw
        chunk_rows.append(rows)
        chunk_base.append(base)
        base += N_CORES * rows
    return group_starts, chunk_rows, chunk_base


def node_pos(nchunks, gw=GW):
    """Position of each node in the chunked zfull layout ([N] int64)."""
    if nchunks <= 1:
        return np.arange(N, dtype=np.int64)
    group_starts, chunk_rows, chunk_base = chunk_layout(nchunks, gw)
    n = np.arange(N, dtype=np.int64)
    c = n // SHARD
    r = n % SHARD
    pos = np.zeros(N, np.int64)
    for j, gs in enumerate(group_starts):
        lo = gs * gw
        hi = lo + chunk_rows[j]
        m = (r >= lo) & (r < hi)
        pos[m] = chunk_base[j] + c[m] * chunk_rows[j] + (r[m] - lo)
    return pos


def prep_graph(edge_index, ipc=IDX_PER_CALL, gw=GW, nchunks=None):
    if nchunks is None:
        nchunks = BUILD_KW.get("nchunks", 1)
    n_groups = math.ceil(SHARD / gw)
    src = edge_index[0].astype(np.int64)
    dst = edge_index[1].astype(np.int64)
    deg = (np.bincount(dst, minlength=N) + 1).astype(np.float64)  # +1 self
    dinv = 1.0 / np.sqrt(deg)
    norm = (dinv[src] * dinv[dst]).astype(np.float32)
    pos = node_pos(nchunks, gw)
    src = pos[src]              # gather by table position, not node id

    core = dst // SHARD
    gloc = (dst % SHARD) // gw
    half = (src >= HALF).astype(np.int64)
    cell = (core * n_groups + gloc) * 2 + half

    counts = np.bincount(cell, minlength=N_CORES * n_groups * 2)
    counts = counts.reshape(N_CORES, n_groups, 2)
    B = np.ceil(counts / BLK).astype(np.int64).max(axis=0)   # [N_GROUPS, 2]

    # per-half streams; cell (g,h) occupies B[g,h]*BLK slots of stream h
    stream_blocks = [B[:, h].sum() for h in (0, 1)]
    ncalls = [math.ceil(sb * BLK / ipc) for sb in stream_blocks]
    stream_slots = [nc_ * ipc for nc_ in ncalls]
    cell_base = np.zeros((n_groups, 2), np.int64)           # slot base within stream h
    for h in (0, 1):
        cell_base[:, h] = np.cumsum(B[:, h] * BLK) - B[:, h] * BLK

    # rank of each edge within its cell; secondary sort by src so the
    # gather's DMA descriptors read ascending addresses (HBM row-buffer
    # locality)
    order = np.lexsort((src, cell))
    cell_sorted = cell[order]
    starts = np.searchsorted(cell_sorted, np.arange(N_CORES * n_groups * 2))
    rank = np.arange(cell.shape[0]) - starts[cell_sorted]
    # slot within the edge's (core, stream-h): cell_base + rank
    g_s = gloc[order]
    h_s = half[order]
    c_s = core[order]
    slot = cell_base[g_s, h_s] + rank

    idx16 = (src[order] - h_s * HALF).astype(np.int16)
    dstloc = ((dst[order] % SHARD) % gw).astype(np.float32)
    normv = norm[order].astype(np.float32)

    # self-loop diag norms: [128, n_groups*2] per core (col = 2*g + t)
    dinv2 = (dinv * dinv).astype(np.float32)
    nself_cols = n_groups * math.ceil(gw / 128)
    per_core = []
    NBs = [sl // BLK for sl in stream_slots]
    for c in range(N_CORES):
        m = c_s == c
        data = {}
        for h in (0, 1):
            mh = m & (h_s == h)
            idx_flat = np.zeros(stream_slots[h], np.int16)
            dl_flat = np.zeros(stream_slots[h], np.float32)
            nm_flat = np.zeros(stream_slots[h], np.float32)
            s = slot[mh]
            idx_flat[s] = idx16[mh]
            dl_flat[s] = dstloc[mh]
            nm_flat[s] = normv[mh]
            data[f"idx{h}"] = _wrap_idx(idx_flat)
            data[f"dl{h}"] = dl_flat.reshape(NBs[h], BLK).T.copy()   # [128, NB_h]
            data[f"nm{h}"] = nm_flat.reshape(NBs[h], BLK).T.copy()
        nms = np.zeros((128, nself_cols), np.float32)
        for g in range(n_groups):
            for t in range(math.ceil(gw / 128)):
                base = c * SHARD + g * gw + t * 128
                nrows = min(128, max(0, SHARD - (g * gw + t * 128)))
                if nrows > 0:
                    nms[:nrows, 2 * g + t] = dinv2[base:base + nrows]
        data["nmself"] = nms
        per_core.append(data)
    return B, ncalls, NBs, per_core


# ---------------------------------------------------------------- bass kernel

def build_nc(B, ncalls, NBs, ablate=(), reps=1, nq=4, sp=False,
             ipc=IDX_PER_CALL, gbufs=8, gw=GW, deep=True, scratch=49152,
             nchunks=1):
    n_groups = math.ceil(SHARD / gw)
    tpg = math.ceil(gw / 128)          # self blocks per (full) group
    group_starts, chunk_rows, chunk_base = chunk_layout(nchunks, gw)
    last_group_of_chunk = {min(gs + math.ceil(n_groups / nchunks), n_groups) - 1: j
                           for j, gs in enumerate(group_starts)}
    """ablate: subset of {"gather", "onehot", "matmul", "collective", "dense"}
    — drop that phase (wrong results, used for perf bisection only).
    scratch: SWDGE descriptor carveout bytes/partition; ring capacity per
    queue is scratch//16 descs — must exceed ipc for gen/transfer overlap."""
    nc = bacc.Bacc("TRN2", target_bir_lowering=False, debug=False,
                   num_devices=N_CORES, num_swdge_queues=nq,
                   dynamic_dma_scratch_size=scratch)

    x_tab = nc.dram_tensor("x_tab", [N, D], F16, kind="ExternalInput")
    xshard_in = nc.dram_tensor("xshard", [SHARD_PAD, D], F16,
                               kind="ExternalInput")
    idx_in = [nc.dram_tensor(f"idx{h}", [128, ncalls[h] * ipc // 16], I16,
                             kind="ExternalInput") for h in (0, 1)]
    dl_in = [nc.dram_tensor(f"dl{h}", [128, NBs[h]], F32, kind="ExternalInput")
             for h in (0, 1)]
    nm_in = [nc.dram_tensor(f"nm{h}", [128, NBs[h]], F32, kind="ExternalInput")
             for h in (0, 1)]
    nmself_in = nc.dram_tensor("nmself", [128, n_groups * tpg], F32,
                               kind="ExternalInput")
    dlself_in = nc.dram_tensor("dlself", [128, tpg], F32, kind="ExternalInput")
    iota_in = nc.dram_tensor("iota", [128, gw], F16, kind="ExternalInput")
    id16_in = nc.dram_tensor("id16", [128, 128], F16, kind="ExternalInput")
    id32_in = nc.dram_tensor("id32", [128, 128], F32, kind="ExternalInput")
    w_in = [nc.dram_tensor(f"w{l}t", [D, D], F16, kind="ExternalInput")
            for l in range(3)]
    b_in = [nc.dram_tensor(f"b{l}", [128, 1], F32, kind="ExternalInput")
            for l in range(3)]
    y_out = nc.dram_tensor("y", [SHARD, D], F32, kind="ExternalOutput")

    zshard = [nc.dram_tensor(f"z{l}s", [SHARD_PAD, D], F16) for l in range(2)]
    zfull = [nc.dram_tensor(f"z{l}f", [N, D], F16, addr_space="Shared")
             for l in range(2)]

    with tile.TileContext(nc) as tc:
        with tc.tile_pool(name="const", bufs=1) as cpool, \
             tc.tile_pool(name="gp", bufs=2 * gbufs) as g_pool, \
             tc.tile_pool(name="sl", bufs=12) as sl_pool, \
             tc.tile_pool(name="s", bufs=(32 if deep else 4)) as s_pool, \
             tc.tile_pool(name="a", bufs=(4 if deep else 2)) as a_pool, \
             tc.tile_pool(name="z", bufs=(4 if deep else 2)) as z_pool, \
             tc.tile_pool(name="t", bufs=4) as t_pool, \
             tc.tile_pool(name="psa", bufs=(4 if deep else 2), space="PSUM") as psa_pool, \
             tc.tile_pool(name="pso", bufs=2, space="PSUM") as pso_pool, \
             tc.tile_pool(name="pst", bufs=2, space="PSUM") as pst_pool:

            # ---- constants
            idx_t, dl_t, nm_t = [], [], []
            for h in (0, 1):
                it = cpool.tile([128, ncalls[h] * ipc // 16], I16, tag=f"idx{h}")
                nc.sync.dma_start(out=it[:], in_=idx_in[h][:, :])
                idx_t.append(it)
                dt_ = cpool.tile([128, NBs[h]], F32, tag=f"dl{h}")
                nc.sync.dma_start(out=dt_[:], in_=dl_in[h][:, :])
                dl_t.append(dt_)
                nt = cpool.tile([128, NBs[h]], F32, tag=f"nm{h}")
                nc.sync.dma_start(out=nt[:], in_=nm_in[h][:, :])
                nm_t.append(nt)
            nmself_t = cpool.tile([128, n_groups * tpg], F32, tag="nmself")
            nc.sync.dma_start(out=nmself_t[:], in_=nmself_in[:, :])
            dlself_t = cpool.tile([128, tpg], F32, tag="dlself")
            nc.sync.dma_start(out=dlself_t[:], in_=dlself_in[:, :])
            iota_t = cpool.tile([128, gw], F16, tag="iota")
            nc.sync.dma_start(out=iota_t[:], in_=iota_in[:, :])
            id16_t = cpool.tile([128, 128], F16, tag="id16")
            nc.sync.dma_start(out=id16_t[:], in_=id16_in[:, :])
            id32_t = cpool.tile([128, 128], F32, tag="id32")
            nc.sync.dma_start(out=id32_t[:], in_=id32_in[:, :])
            w_t, b_t = [], []
            for l in range(3):
                wt = cpool.tile([D, D], F16, tag=f"w{l}")
                nc.sync.dma_start(out=wt[:], in_=w_in[l][:, :])
                w_t.append(wt)
                bt = cpool.tile([128, 1], F32, tag=f"b{l}")
                nc.sync.dma_start(out=bt[:], in_=b_in[l][:, :])
                b_t.append(bt)

            # zero the zshard pad rows once (self-loop blocks over-read them;
            # nm=0 kills the contribution but the data must be finite)
            zpad = cpool.tile([128, D], F16, tag="zpad")
            nc.vector.memset(zpad[:], 0.0)
            for l in range(2):
                nc.sync.dma_start(out=zshard[l][SHARD:SHARD_PAD, :],
                                  in_=zpad[:SHARD_PAD - SHARD, :])

            s_zero = None
            if "onehot" in ablate:
                s_zero = cpool.tile([128, gw], F16, tag="szero")
                nc.vector.memset(s_zero[:], 0.0)

            # block -> stream slot base for each (g, h)
            cell_base_blk = np.zeros((n_groups, 2), np.int64)
            for h in (0, 1):
                cell_base_blk[:, h] = np.cumsum(B[:, h]) - B[:, h]

            for rep_layer in range(3 * reps):
                layer = rep_layer % 3
                z_tab = [x_tab, zfull[0], zfull[1]][layer]
                z_self = [xshard_in, zshard[0], zshard[1]][layer]
                tabs = [z_tab[0:HALF, :], z_tab[HALF:N, :]]
                g_tiles = [{}, {}]   # per stream: call -> tile

                def get_block(h, blk_i):
                    call = (blk_i * BLK) // ipc
                    j = blk_i - call * (ipc // BLK)
                    if call not in g_tiles[h]:
                        gt = g_pool.tile([128, ipc // BLK, D], F16,
                                         tag="g")
                        if "gather" not in ablate:
                            nc.gpsimd.dma_gather(
                                out_ap=gt[:],
                                in_ap=tabs[h],
                                idxs_ap=idx_t[h][:, call * (ipc // 16):
                                                 (call + 1) * (ipc // 16)],
                                num_idxs=ipc,
                                num_idxs_reg=ipc,
                                elem_size=D,
                                queue_num=(h + 2 * call) % nq,
                                single_packet=sp,
                            )
                        else:
                            nc.vector.memset(gt[:, 0, :], 0.25)
                        g_tiles[h][call] = gt
                    return g_tiles[h][call][:, j, :]

                for g in range(n_groups):
                    gw_act = min(gw, SHARD - g * gw)
                    nself = math.ceil(gw_act / 128)
                    nblk = int(B[g, 0] + B[g, 1]) + nself
                    psA = psa_pool.tile([128, gw], F32, tag="psa")
                    bi = 0
                    # self-loop blocks first (sequential dma, warms PE)
                    for t in range(nself):
                        base = g * gw + t * 128
                        st = sl_pool.tile([128, 128], F16, tag="slf")
                        if "gather" not in ablate:
                            nc.sync.dma_start(
                                out=st[:],
                                in_=z_self[base:base + 128, :],
                            )
                        else:
                            nc.vector.memset(st[:, 0:128], 0.25)
                        if "onehot" not in ablate:
                            sT = s_pool.tile([128, gw], F16, tag="s")
                            nc.vector.tensor_scalar(
                                out=sT[:], in0=iota_t[:],
                                scalar1=dlself_t[:, t:t + 1],
                                scalar2=nmself_t[:, tpg * g + t:tpg * g + t + 1],
                                op0=mybir.AluOpType.is_equal,
                                op1=mybir.AluOpType.mult,
                            )
                        else:
                            sT = s_zero
                        stop_now = (bi == nblk - 1)
                        if "matmul" not in ablate:
                            nc.tensor.matmul(
                                out=psA[:], lhsT=st[:], rhs=sT[:],
                                start=(bi == 0), stop=stop_now,
                            )
                        elif bi == 0:
                            nc.tensor.matmul(out=psA[:], lhsT=st[:], rhs=sT[:],
                                             start=True, stop=True)
                        bi += 1
                    for h in (0, 1):
                        for i in range(int(B[g, h])):
                            blk_i = int(cell_base_blk[g, h] + i)
                            gblk = get_block(h, blk_i)
                            if "onehot" not in ablate:
                                sT = s_pool.tile([128, gw], F16, tag="s")
                                nc.vector.tensor_scalar(
                                    out=sT[:], in0=iota_t[:],
                                    scalar1=dl_t[h][:, blk_i:blk_i + 1],
                                    scalar2=nm_t[h][:, blk_i:blk_i + 1],
                                    op0=mybir.AluOpType.is_equal,
                                    op1=mybir.AluOpType.mult,
                                )
                            else:
                                sT = s_zero
                            if "matmul" not in ablate:
                                nc.tensor.matmul(
                                    out=psA[:], lhsT=gblk, rhs=sT[:],
                                    start=False, stop=(bi == nblk - 1),
                                )
                            bi += 1

                    aT = a_pool.tile([128, gw], F16, tag="a")
                    nc.scalar.activation(out=aT[:], in_=psA[:],
                                         func=mybir.ActivationFunctionType.Copy)
                    psO = pso_pool.tile([128, gw], F32, tag="pso")
                    nc.tensor.matmul(out=psO[:], lhsT=w_t[layer][:], rhs=aT[:],
                                     start=True, stop=True)

                    if layer < 2:
                        zT = z_pool.tile([128, gw], F16, tag="z16")
                        nc.scalar.activation(out=zT[:], in_=psO[:],
                                             func=mybir.ActivationFunctionType.Relu,
                                             bias=b_t[layer][:])
                        ident = id16_t
                        odt = F16
                        dest = zshard[layer]
                    else:
                        zT = z_pool.tile([128, gw], F32, tag="z32")
                        nc.scalar.activation(out=zT[:], in_=psO[:],
                                             func=mybir.ActivationFunctionType.Identity,
                                             bias=b_t[layer][:])
                        ident = id32_t
                        odt = F32
                        dest = y_out

                    for t in range(math.ceil(gw_act / 128)):
                        width = min(128, gw_act - t * 128)
                        psT = pst_pool.tile([128, 128], odt, tag="pst")
                        nc.tensor.transpose(out=psT[:], in_=zT[:, t * 128:(t + 1) * 128],
                                            identity=ident[:])
                        ts_ = t_pool.tile([128, 128], odt, tag="t")
                        nc.scalar.activation(out=ts_[:], in_=psT[:],
                                             func=mybir.ActivationFunctionType.Copy)
                        base = g * gw + t * 128
                        nc.sync.dma_start(out=dest[base:base + width, :],
                                          in_=ts_[:width, :])

                    if layer < 2 and g in last_group_of_chunk:
                        j = last_group_of_chunk[g]
                        r0 = group_starts[j] * gw
                        rj = chunk_rows[j]
                        bj = chunk_base[j]
                        if "collective" in ablate:
                            for c in range(N_CORES):
                                nc.sync.dma_start(
                                    out=zfull[layer][bj + c * rj:
                                                     bj + (c + 1) * rj, :],
                                    in_=zshard[layer][r0:r0 + rj, :])
                        else:
                            nc.gpsimd.collective_compute(
                                "AllGather", mybir.AluOpType.bypass,
                                replica_groups=[list(range(N_CORES))],
                                ins=[zshard[layer][r0:r0 + rj, :].opt()],
                                outs=[zfull[layer][bj:bj + N_CORES * rj, :].opt()],
                            )

    nc.compile()
    return nc


class _MakespanFilter(logging.Filter):
    """Captures the Tile scheduling sim's predicted makespan."""

    def __init__(self):
        super().__init__()
        self.times = []

    def filter(self, record):
        m = re.search(r"Simulation completed at time (\d+)", record.getMessage())
        if m:
            self.times.append(int(m.group(1)))
        return True


def build_with_makespan(*args, **kwargs):
    lg = logging.getLogger("concourse")
    old_level = lg.level
    f = _MakespanFilter()
    lg.addFilter(f)
    lg.setLevel(logging.DEBUG)
    try:
        nc = build_nc(*args, **kwargs)
    finally:
        lg.removeFilter(f)
        lg.setLevel(old_level)
    makespan = max(f.times) if f.times else None
    return nc, makespan


# ---------------------------------------------------------------- runner

class SpmdRunner:
    """Persistent jitted SPMD executor (axon/PJRT path, jit built once)."""

    def __init__(self, nc, n_cores):
        import jax
        from jax.sharding import Mesh, PartitionSpec
        from jax.experimental.shard_map import shard_map
        from concourse.bass2jax import (_bass_exec_p, install_neuronx_cc_hook,
                                        partition_id_tensor)
        install_neuronx_cc_hook()
        self.jax = jax
        self.nc = nc
        self.n_cores = n_cores
        partition_name = nc.partition_id_tensor.name if nc.partition_id_tensor else None
        in_names, out_names, out_avals, zero_outs = [], [], [], []
        for alloc in nc.m.functions[0].allocations:
            if not isinstance(alloc, mybir.MemoryLocationSet):
                continue
            name = alloc.memorylocations[0].name
            if alloc.kind == "ExternalInput":
                if name != partition_name:
                    in_names.append(name)
            elif alloc.kind == "ExternalOutput":
                shape = tuple(alloc.tensor_shape)
                dtype = mybir.dt.np(alloc.dtype)
                out_names.append(name)
                out_avals.append(jax.core.ShapedArray(shape, dtype))
                zero_outs.append(np.zeros(shape, dtype))
        self.in_names, self.out_names = in_names, out_names
        self.out_avals, self.zero_outs = out_avals, zero_outs
        n_params, n_outs = len(in_names), len(out_avals)
        all_in = list(in_names) + list(out_names)
        if partition_name is not None:
            all_in.append(partition_name)

        def _body(*args):
            operands = list(args)
            if partition_name is not None:
                operands.append(partition_id_tensor())
            outs = _bass_exec_p.bind(
                *operands, out_avals=tuple(out_avals), in_names=tuple(all_in),
                out_names=tuple(out_names), lowering_input_output_aliases=(),
                sim_require_finite=True, sim_require_nnan=True, nc=nc)
            return tuple(outs)

        devices = jax.devices()[:n_cores]
        mesh = Mesh(np.asarray(devices), ("core",))
        from jax.sharding import NamedSharding
        from jax.sharding import PartitionSpec as P
        self._sharding = NamedSharding(mesh, P("core"))
        self._fn = jax.jit(
            shard_map(_body, mesh=mesh,
                      in_specs=(P("core"),) * (n_params + n_outs),
                      out_specs=(P("core"),) * n_outs, check_rep=False),
            keep_unused=True)
        self._staged = None

    def stage_inputs(self, in_maps):
        n = self.n_cores
        concat = [np.concatenate([np.asarray(in_maps[c][nm]) for c in range(n)], axis=0)
                  for nm in self.in_names]
        concat += [np.zeros((n * z.shape[0], *z.shape[1:]), z.dtype)
                   for z in self.zero_outs]
        self._staged = [self.jax.device_put(a, self._sharding) for a in concat]

    def run(self):
        outs = self._fn(*self._staged)
        self.jax.block_until_ready(outs)
        return outs

    def results(self, outs):
        res = []
        for c in range(self.n_cores):
            m = {}
            for i, nm in enumerate(self.out_names):
                full = np.asarray(outs[i])
                m[nm] = full.reshape(self.n_cores, *self.out_avals[i].shape)[c]
            res.append(m)
        return res


_CACHE = {}

# default build configuration (tuned)
BUILD_KW = dict(sp=True, nq=4, gbufs=40, scratch=16384, nchunks=3)


def _get_built(B_key, B, ncalls, NBs, reps=1):
    key = (B_key, reps)
    if key not in _CACHE:
        nc, makespan = build_with_makespan(B, ncalls, NBs, reps=reps,
                                           **BUILD_KW)
        if makespan:
            print(f"[kernel] predicted makespan: {makespan} ns")
        _CACHE[key] = (nc, SpmdRunner(nc, N_CORES))
    return _CACHE[key]


def build_in_maps(inputs, per_core, nchunks=None):
    """Per-core input dicts from the full-problem input dict."""
    if nchunks is None:
        nchunks = BUILD_KW.get("nchunks", 1)
    x16n = np.asarray(inputs["x"]).astype(np.float16)
    x16 = np.empty_like(x16n)
    x16[node_pos(nchunks)] = x16n          # chunked table layout
    iota = np.tile(np.arange(GW, dtype=np.float16), (128, 1))
    ident = np.eye(128)
    tpg = math.ceil(GW / 128)
    dlself = np.zeros((128, tpg), np.float32)
    for t in range(tpg):
        dlself[:, t] = t * 128 + np.arange(128)
    xsh = np.zeros((N_CORES, SHARD_PAD, D), np.float16)
    xsh[:, 0:SHARD] = x16n.reshape(N_CORES, SHARD, D)
    common = {
        "x_tab": x16,
        "iota": iota,
        "dlself": dlself,
        "id16": ident.astype(np.float16),
        "id32": ident.astype(np.float32),
        "w0t": np.asarray(inputs["W1"]).T.astype(np.float16),
        "w1t": np.asarray(inputs["W2"]).T.astype(np.float16),
        "w2t": np.asarray(inputs["W3"]).T.astype(np.float16),
        "b0": np.asarray(inputs["b1"]).reshape(128, 1).astype(np.float32),
        "b1": np.asarray(inputs["b2"]).reshape(128, 1).astype(np.float32),
        "b2": np.asarray(inputs["b3"]).reshape(128, 1).astype(np.float32),
    }
    return [{**common, "xshard": xsh[c], **per_core[c]} for c in range(N_CORES)]


def kernel(x, edge_index, W1, b1, W2, b2, W3, b3):
    x = np.asarray(x)
    edge_index = np.asarray(edge_index)
    nchunks = BUILD_KW.get("nchunks", 1)
    B, ncalls, NBs, per_core = prep_graph(edge_index, nchunks=nchunks)
    B_key = (tuple(B.flatten().tolist()), tuple(ncalls))
    nc, runner = _get_built(B_key, B, ncalls, NBs)
    in_maps = build_in_maps(
        {"x": x, "W1": W1, "b1": b1, "W2": W2, "b2": b2, "W3": W3, "b3": b3},
        per_core, nchunks=nchunks)
    runner.stage_inputs(in_maps)
    outs = runner.run()
    res = runner.results(outs)
    return np.concatenate([res[c]["y"] for c in range(N_CORES)], axis=0)
